# revision 34
# baseline (speedup 1.0000x reference)
"""Bahdanau attention scores kernel for Trainium2 (8 NeuronCores).

Math (per batch row b):
    energy[s, :] = tanh(hidden[b] @ W_h + enc[s, b, :] @ W_e + bias)
    scores[s]    = energy[s, :] . v
    out[b, :]    = softmax(scores)

Strategy:
  - Data-parallel: batch (32) sharded 4-per-core across 8 cores; weights
    replicated. No cross-core communication (softmax is per-row over seq).
  - The big matmul (enc @ W_e, contraction over the 1024 encoder features)
    runs in fp16 (measured end-to-end L2 rel err ~2e-3 vs fp32): fp16 streams
    at full PE rate and, unlike fp32, supports DMA-xbar transposes.
  - enc arrives [S, B, E]; the PE needs E on partitions. enc tiles are
    cast fp32->fp16 during the HBM->SBUF DMA (SWDGE), then transposed
    SBUF->SBUF via the DMA xbar (no PE/DVE cycles).
  - energy^T tiles [d, s] accumulate in PSUM; ACT applies tanh with the
    per-(batch, d) bias hWh^T = W_h^T @ hidden^T + b as the activation bias,
    writing fp16; a second PE matmul with v as the 1-column stationary
    reduces over d into scores [1, 512] chunks.
  - Softmax per batch on [1, 2048]: -max via negated reduce, exp with
    accumulated sum on ACT, reciprocal + scale on DVE.
"""

import sys

for _p in ("/opt/trn_rl_repo", "/root/.axon_site/_ro/trn_rl_repo"):
    if _p not in sys.path:
        sys.path.append(_p)

from contextlib import ExitStack

import numpy as np

import concourse.bass as bass
import concourse.tile as tile
from concourse import mybir
from concourse.bass_utils import run_bass_kernel_spmd

P = 128
S, B, E, D = 2048, 32, 1024, 1024  # seq, batch, 2*enc_hs, dec_hs
NCORES = 8
BL = B // NCORES  # batches per core
ST = 512  # seq rows per tile
NST = S // ST
EC = E // P  # 8 e-chunks
DC = D // P  # 8 d-chunks
WT = ST // P  # 4 transpose blocks per seq tile

f32 = mybir.dt.float32
f16 = mybir.dt.float16


def _split_multiwaits(nc):
    """This container's walrus rejects >1 semaphore wait per instruction
    ("Too many sync wait commands"); Tile attaches several to its final
    drain. Move extra waits onto dedicated NoOps just before the carrying
    instruction (same engine, program order => identical blocking)."""
    for fn in nc.m.functions:
        for bb in fn.blocks:
            out = []
            changed = False
            for inst in bb.instructions:
                si = inst.sync_info
                waits = list(si.on_wait) if si is not None and si.on_wait else []
                limit = 0 if isinstance(inst, mybir.InstDrain) else 1
                if len(waits) > limit:
                    for w in waits[limit:]:
                        out.append(
                            mybir.InstNoOp(
                                name=nc.get_next_instruction_name(),
                                opcode="NoOp",
                                engine=inst.engine,
                                sync_info=mybir.SyncInfo(on_wait=[w], on_update=[]),
                                text_hint="waitfix",
                                bass_nofuse=True,
                            )
                        )
                    si.on_wait = waits[:limit]
                    changed = True
                out.append(inst)
            if changed:
                bb.instructions.clear()
                for inst in out:
                    bb.instructions.append(inst)


def _build():
    nc = bass.Bass()
    enc = nc.declare_dram_parameter("enc", [BL, S, E], f32, isOutput=False)
    hid = nc.declare_dram_parameter("hidden", [BL, D], f32, isOutput=False)
    w = nc.declare_dram_parameter("attn_w", [E + D, D], f32, isOutput=False)
    bvec = nc.declare_dram_parameter("attn_b", [D], f32, isOutput=False)
    vvec = nc.declare_dram_parameter("v", [D], f32, isOutput=False)
    out = nc.declare_dram_parameter("out", [BL, S], f32, isOutput=True)

    with tile.TileContext(nc) as tc, ExitStack() as ctx:
        consts = ctx.enter_context(tc.tile_pool(name="consts", bufs=1))
        encp = ctx.enter_context(tc.tile_pool(name="encp", bufs=3))
        enctp = ctx.enter_context(tc.tile_pool(name="enctp", bufs=3))
        tanhp = ctx.enter_context(tc.tile_pool(name="tanhp", bufs=2))
        smp = ctx.enter_context(tc.tile_pool(name="smp", bufs=2))
        psumE = ctx.enter_context(tc.tile_pool(name="psumE", bufs=3, space="PSUM"))
        psumS = ctx.enter_context(tc.tile_pool(name="psumS", bufs=2, space="PSUM"))
        psumH = ctx.enter_context(tc.tile_pool(name="psumH", bufs=1, space="PSUM"))

        # ---- constants -------------------------------------------------
        wT = w.rearrange("(half ko p) d -> half p ko d", half=2, p=P)
        Wh16 = consts.tile([P, DC, D], f16)
        We16 = consts.tile([P, EC, D], f16)

        def load_enc_tile(b, st, split=False):
            # stage enc rows [st*512 .. +512) of batch b, cast to fp16; the
            # first tile is loaded in four 512KB pieces so its transposes
            # start before the full 2MB lands
            enc_nat = encp.tile([P, WT, E], f16)
            src = enc[b, st * ST : (st + 1) * ST, :].rearrange(
                "(w p) e -> p w e", p=P
            )
            encT = enctp.tile([P, EC, ST], f16)
            for wdx in range(WT):
                if split:
                    nc.gpsimd.dma_start(
                        out=enc_nat[:, wdx, :], in_=src[:, wdx, :]
                    )
                elif wdx == 0:
                    nc.gpsimd.dma_start(out=enc_nat[:], in_=src)
                nc.sync.dma_start_transpose(
                    encT[:, :, wdx * P : (wdx + 1) * P], enc_nat[:, wdx, :]
                )
            return encT

        tiles = [(b, st) for b in range(BL) for st in range(NST)]
        encTs = {}
        # All heavy loads share the 16 SDMA engines; SWDGE FIFO order is the
        # only real priority control. Order: hidden/bias/v combo (tiny),
        # enc0 in pieces (feeds the transpose pipeline immediately), W_h
        # (the hWh bias chain must beat the first tanh), W_e chunks (the
        # first psE group trickles in as they land), then enc1..2.
        hb_nat = consts.tile([16, D], f16)
        nc.gpsimd.dma_start(out=hb_nat[0:BL, :], in_=hid[:, :])
        nc.gpsimd.dma_start(
            out=hb_nat[BL : BL + 1, :], in_=bvec.rearrange("(o d) -> o d", o=1)
        )
        nc.gpsimd.dma_start(
            out=hb_nat[BL + 1 : BL + 2, :], in_=vvec.rearrange("(o d) -> o d", o=1)
        )
        hbT = consts.tile([P, DC, 16], f16)
        nc.sync.dma_start_transpose(hbT[:], hb_nat[:])
        encTs[0] = load_enc_tile(*tiles[0], split=True)
        nc.gpsimd.dma_start(out=Wh16[:, :4, :], in_=wT[0, :, :4, :])
        nc.gpsimd.dma_start(out=Wh16[:, 4:, :], in_=wT[0, :, 4:, :])
        for ec in range(EC):
            nc.gpsimd.dma_start(out=We16[:, ec, :], in_=wT[1, :, ec, :])
        encTs[1] = load_enc_tile(*tiles[1])
        encTs[2] = load_enc_tile(*tiles[2])

        # ---- hWh^T = W_h^T @ hidden^T + b : [d, batch] -----------------
        bT32 = consts.tile([P, DC], f32)
        nc.vector.tensor_copy(out=bT32[:], in_=hbT[:, :, BL])
        hwhb = consts.tile([P, DC, BL], f32)
        for dc in range(DC):
            ps = psumH.tile([P, BL], f32, tag="pshwh")
            for hc in range(DC):
                nc.tensor.matmul(
                    ps[:],
                    Wh16[:, hc, dc * P : (dc + 1) * P],
                    hbT[:, hc, :BL],
                    start=(hc == 0),
                    stop=(hc == DC - 1),
                )
            nc.vector.tensor_scalar_add(
                out=hwhb[:, dc, :], in0=ps[:], scalar1=bT32[:, dc : dc + 1]
            )

        # ---- main loop -------------------------------------------------
        for b in range(BL):
            scores = smp.tile([1, S], f32, tag="scores")
            for st in range(NST):
                i = b * NST + st
                if i + 2 < len(tiles) and (i + 2) not in encTs:
                    encTs[i + 2] = load_enc_tile(*tiles[i + 2])
                encT = encTs.pop(i)
                th = tanhp.tile([P, DC, ST], f16, tag="th")
                for dc in range(DC):
                    psE = psumE.tile([P, ST], f32)
                    for ec in range(EC):
                        nc.tensor.matmul(
                            psE[:],
                            We16[:, ec, dc * P : (dc + 1) * P],
                            encT[:, ec, :],
                            start=(ec == 0),
                            stop=(ec == EC - 1),
                        )
                    nc.scalar.activation(
                        th[:, dc, :],
                        psE[:],
                        mybir.ActivationFunctionType.Tanh,
                        bias=hwhb[:, dc, b : b + 1],
                    )
                # batched v-dot on PE: one stationary-swap per row-tile
                psS = psumS.tile([1, ST], f32, tag="psS")
                for dc in range(DC):
                    nc.tensor.matmul(
                        psS[:],
                        hbT[:, dc, BL + 1 : BL + 2],
                        th[:, dc, :],
                        start=(dc == 0),
                        stop=(dc == DC - 1),
                        skip_group_check=True,
                    )
                nc.vector.tensor_copy(
                    out=scores[:, st * ST : (st + 1) * ST], in_=psS[:]
                )
            # ---- softmax over S on partition 0 -------------------------
            negmx = smp.tile([1, 1], f32, tag="negmx")
            nc.vector.tensor_reduce(
                out=negmx[:],
                in_=scores[:],
                axis=mybir.AxisListType.X,
                op=mybir.AluOpType.max,
                negate=True,
            )
            probs = smp.tile([1, S], f32, tag="probs")
            ssum = smp.tile([1, 1], f32, tag="ssum")
            nc.scalar.activation(
                probs[:],
                scores[:],
                mybir.ActivationFunctionType.Exp,
                bias=negmx[:],
                accum_out=ssum[:],
            )
            rec = smp.tile([1, 1], f32, tag="rec")
            nc.vector.reciprocal(out=rec[:], in_=ssum[:])
            nc.vector.tensor_scalar_mul(out=probs[:], in0=probs[:], scalar1=rec[:])
            nc.sync.dma_start(out=out[b, :], in_=probs[:])

    _split_multiwaits(nc)
    return nc


_NC = None


def _get_nc():
    global _NC
    if _NC is None:
        _NC = _build()
    return _NC


def kernel(hidden, encoder_outputs, attn_w, attn_b, v):
    nc = _get_nc()
    hidden = np.ascontiguousarray(hidden, dtype=np.float32)
    attn_w = np.ascontiguousarray(attn_w, dtype=np.float32)
    attn_b = np.ascontiguousarray(attn_b, dtype=np.float32)
    v = np.ascontiguousarray(v, dtype=np.float32)
    in_maps = []
    for c in range(NCORES):
        in_maps.append(
            {
                "enc": np.ascontiguousarray(
                    encoder_outputs[:, c * BL : (c + 1) * BL, :].transpose(1, 0, 2),
                    dtype=np.float32,
                ),
                "hidden": np.ascontiguousarray(hidden[c * BL : (c + 1) * BL]),
                "attn_w": attn_w,
                "attn_b": attn_b,
                "v": v,
            }
        )
    res = run_bass_kernel_spmd(nc, in_maps, core_ids=list(range(NCORES)))
    return np.concatenate(
        [res.results[c]["out"] for c in range(NCORES)], axis=0
    ).astype(np.float32)
